# revision 30
# baseline (speedup 1.0000x reference)
"""Cross-attention layer with 3D RoPE on 8 Trainium2 NeuronCores.

Sharding: core c -> (batch b = c//2, head-group hg = c%2 of 4 heads).
Each core computes its batch's partial output projection for its 4 heads;
the host sums the two half-partials per batch and adds the output bias.

Layout: all heavy tensors are bf16; token/weight inputs are host-packed into
per-partition-concatenated blocks so every big DMA is one fully contiguous
>=0.5MB transfer.  No zero padding: matmuls contract over 96 (head dim) or 3
(coords) partitions directly.

Scheduling: q/k projection skews the rope ops one head behind the projection
matmuls so PE never head-of-line blocks on the ACT bias-add.  Attention runs
k-group-major (all 8 (jq,h) streams advance one score-group at a time) with
the softmax accumulator in SBUF, which lets the v-projection be interleaved
into the attention pipeline (PE does v-proj matmuls while ACT runs exp), and
attn@v weight loads are skewed one group behind the score matmuls.
"""

import sys

import numpy as np
import ml_dtypes

try:
    import concourse.bass as bass
except ImportError:  # fresh-dir grading: container repo path
    sys.path.insert(0, "/opt/trn_rl_repo")
    import concourse.bass as bass

import concourse.tile as tile
from concourse import bacc, bass_utils, mybir

F32 = mybir.dt.float32
R32 = mybir.dt.float32r  # full-rate fp32 matmul mode (1 cyc/row at N>=256)
BF = mybir.dt.bfloat16
BF_NP = ml_dtypes.bfloat16

B, Lq, Lk, D, H = 4, 1024, 4096, 768, 8
HD = 96            # head dim
HL = 4             # heads per core
DL = HL * HD       # 384 local d per core
N_CORES = 8
SCALE = 1.0 / float(np.sqrt(np.float32(HD)))
PI = float(np.pi)

NC6 = D // 128     # 6 c-tiles of the contraction dim
NJQ = Lq // 512    # 2 q token tiles
NJK = Lk // 512    # 8 k token tiles
NKT = Lk // 128    # 32 k tiles for attention
GRP = 3            # score k-tiles per exp instruction


def build_program():
    nc = bacc.Bacc("TRN2", target_bir_lowering=False, debug=False)
    AF = mybir.ActivationFunctionType
    ALU = mybir.AluOpType

    # DRAM I/O (per-core shapes); token/weight blocks are host-packed so each
    # row holds the c-tiles concatenated: row (jt*128+p), col (c*512+t).
    dq = nc.dram_tensor("qstg", [NJQ * 128, NC6 * 512], BF, kind="ExternalInput").ap()
    dk = nc.dram_tensor("kstg", [NJK * 128, NC6 * 512], BF, kind="ExternalInput").ap()
    dv = nc.dram_tensor("vstg", [NJK * 128, NC6 * 512], BF, kind="ExternalInput").ap()
    dcq = nc.dram_tensor("cqT", [3, Lq], R32, kind="ExternalInput").ap()
    dck = nc.dram_tensor("ckT", [3, Lk], R32, kind="ExternalInput").ap()
    dwq = nc.dram_tensor("wq", [128, NC6 * DL], BF, kind="ExternalInput").ap()
    dwk = nc.dram_tensor("wk", [128, NC6 * DL], BF, kind="ExternalInput").ap()
    dwv = nc.dram_tensor("wv", [128, NC6 * DL], BF, kind="ExternalInput").ap()
    dwo = nc.dram_tensor("wo", [HD, 4 * D], BF, kind="ExternalInput").ap()
    dbq = nc.dram_tensor("bqh", [HD, HL], F32, kind="ExternalInput").ap()
    dbk = nc.dram_tensor("bkh", [HD, HL], F32, kind="ExternalInput").ap()
    dbv = nc.dram_tensor("bvb", [128, DL], F32, kind="ExternalInput").ap()
    divf = nc.dram_tensor("ivf", [3, HD], R32, kind="ExternalInput").ap()
    dp96 = nc.dram_tensor("p96", [HD, HD], BF, kind="ExternalInput").ap()
    dout = nc.dram_tensor("outT", [D, Lq], F32, kind="ExternalOutput").ap()

    with tile.TileContext(nc) as tc:
        from contextlib import ExitStack

        with ExitStack() as ctx:
            # ---- persistent tensors ----
            big = ctx.enter_context(tc.tile_pool(name="big", bufs=1))
            kT = big.tile([HD, HL, Lk], BF, tag="kT")
            qT = big.tile([HD, HL, Lq], BF, tag="qT")
            vsb = big.tile([128, NKT, HL, HD + 1], BF, tag="vsb")
            o2n = big.tile([HD, HL, Lq], BF, tag="o2n")
            posb = big.tile([HD + 1, NJQ, HL, 512], F32, tag="posb")

            cst = ctx.enter_context(tc.tile_pool(name="cst", bufs=1))
            wq_sb = cst.tile([128, NC6, DL], BF, tag="wq")
            wk_sb = cst.tile([128, NC6, DL], BF, tag="wk")
            wv_sb = cst.tile([128, NC6, DL], BF, tag="wv")
            wo_sb = cst.tile([HD, 4, D], BF, tag="wo")
            ivf = cst.tile([3, HD], R32, tag="ivf")
            p96 = cst.tile([HD, HD], BF, tag="p96")
            bqh = cst.tile([HD, HL], F32, tag="bqh")
            bkh = cst.tile([HD, HL], F32, tag="bkh")
            bvb = cst.tile([128, DL], F32, tag="bvb")

            stage = ctx.enter_context(tc.tile_pool(name="stage", bufs=4))
            cpool = ctx.enter_context(tc.tile_pool(name="cpool", bufs=2))

            def stage_load(src_dram, jt):
                stg = stage.tile([128, NC6, 512], BF, tag="stg")
                rows = slice(jt * 128, (jt + 1) * 128)
                half = (NC6 // 2) * 512
                nc.sync.dma_start(stg[:, 0 : NC6 // 2, :], src_dram[rows, 0:half])
                nc.sync.dma_start(stg[:, NC6 // 2 : NC6, :], src_dram[rows, half:])
                return stg

            # first-needed first: q-path consts and the first q token tile,
            # then the remaining weights
            nc.sync.dma_start(ivf[:], divf[:])
            nc.sync.dma_start(p96[:], dp96[:])
            nc.sync.dma_start(bqh[:], dbq[:])
            nc.sync.dma_start(wq_sb[:], dwq[:])
            stg0 = stage_load(dq, 0)
            nc.sync.dma_start(wk_sb[:], dwk[:])
            nc.sync.dma_start(bkh[:], dbk[:])
            nc.sync.dma_start(wv_sb[:], dwv[:])
            nc.sync.dma_start(bvb[:], dbv[:])
            nc.sync.dma_start(wo_sb[:], dwo[:])

            # ones column of v (row 96 of the attn@v partial = softmax sum)
            nc.vector.memset(vsb[:, :, :, HD : HD + 1], 1.0)

            with ExitStack() as pctx:
                trig = pctx.enter_context(tc.tile_pool(name="trig", bufs=7))
                tmp = pctx.enter_context(tc.tile_pool(name="tmp", bufs=4))
                th_ps = pctx.enter_context(
                    tc.tile_pool(name="th_ps", bufs=2, space="PSUM")
                )
                pj_ps = pctx.enter_context(
                    tc.tile_pool(name="pj_ps", bufs=2, space="PSUM")
                )
                xs_ps = pctx.enter_context(
                    tc.tile_pool(name="xs_ps", bufs=2, space="PSUM")
                )

                # HAM warm-up: dummy matmuls on a zeroed tile keep the PE busy
                # while the first input DMAs land (cold clock is half rate).
                wrm = trig.tile([HD, 512], R32, tag="trig", name="wrm")
                nc.vector.memset(wrm[:].bitcast(F32), 0.0)
                for i in range(12):
                    wps = th_ps.tile([HD, 512], F32, tag="th", name="wps")
                    nc.tensor.matmul(
                        wps[:], wrm[:, 0:HD], wrm[:], start=True, stop=True
                    )

                # ---- q/k projections + rope, rope skewed one head back ----
                pend = []  # (dst, h, ts, sinD, cosD)

                def do_rope():
                    dst, h, ts, sinD, cosD = pend.pop(0)
                    x = dst[:, h, ts]
                    xs = xs_ps.tile([HD, 512], F32, tag="xs")
                    nc.tensor.matmul(xs[:], p96[:], x, start=True, stop=True)
                    m1 = tmp.tile([HD, 512], BF, tag="m1")
                    nc.gpsimd.tensor_mul(m1[:], x, cosD[:])
                    m2 = tmp.tile([HD, 512], BF, tag="m1", name="m2")
                    nc.vector.tensor_mul(m2[:], xs[:], sinD[:])
                    if h % 2 == 0:
                        nc.vector.tensor_add(x, m1[:], m2[:])
                    else:
                        nc.gpsimd.tensor_add(x, m1[:], m2[:])

                def proj_rope(src_dram, coords_dram, w_sb, bias_sb, dst, n_jt):
                    for jt in range(n_jt):
                        ts = slice(jt * 512, (jt + 1) * 512)
                        if src_dram is dq and jt == 0:
                            stg = stg0
                        else:
                            stg = stage_load(src_dram, jt)
                        # theta -> wrapped -> sin/cos (rows: axis*32+half*16+p)
                        cstg = cpool.tile([3, 512], R32, tag="coords")
                        nc.sync.dma_start(cstg[:], coords_dram[:, ts])
                        th = th_ps.tile([HD, 512], F32, tag="th")
                        nc.tensor.matmul(th[:], ivf[:], cstg[:], start=True, stop=True)
                        ws = trig.tile([HD, 512], F32, tag="trig", name="ws")
                        wc = trig.tile([HD, 512], F32, tag="trig", name="wc")
                        nc.vector.add_range_wrap(ws[:], th[:], 0.0, PI, 2 * PI)
                        nc.vector.add_range_wrap(wc[:], th[:], PI / 2, PI, 2 * PI)
                        sinD = trig.tile([HD, 512], BF, tag="trig", name="sin")
                        cosD = trig.tile([HD, 512], BF, tag="trig", name="cos")
                        nc.scalar.activation(sinD[:], ws[:], AF.Sin)
                        nc.scalar.activation(cosD[:], wc[:], AF.Sin)
                        for h in range(HL):
                            ps = pj_ps.tile([HD, 512], F32, tag="pj")
                            for c in range(NC6):
                                nc.tensor.matmul(
                                    ps[:],
                                    w_sb[:, c, h * HD : (h + 1) * HD],
                                    stg[:, c, :],
                                    start=(c == 0),
                                    stop=(c == NC6 - 1),
                                )
                            nc.scalar.activation(
                                dst[:, h, ts], ps[:], AF.Identity,
                                bias=bias_sb[:, h : h + 1],
                            )
                            pend.append((dst, h, ts, sinD, cosD))
                            if len(pend) > 1:
                                do_rope()

                proj_rope(dq, dcq, wq_sb, bqh, qT, NJQ)
                proj_rope(dk, dck, wk_sb, bkh, kT, NJK)
                while pend:
                    do_rope()

            # ---- attention + v-projection + output, one flat pipeline ----
            with ExitStack() as actx:
                s_ps = actx.enter_context(tc.tile_pool(name="s_ps", bufs=2, space="PSUM"))
                pr_ps = actx.enter_context(tc.tile_pool(name="pr_ps", bufs=2, space="PSUM"))
                pt_p = actx.enter_context(tc.tile_pool(name="pt", bufs=3))
                nz_p = actx.enter_context(tc.tile_pool(name="nz", bufs=4))

                def vproj_jt(jt):
                    stg = stage_load(dv, jt)
                    for sub in range(4):
                        kt = jt * 4 + sub
                        ps = pr_ps.tile([128, 512], F32, tag="pr", name="pv")
                        for c in range(NC6):
                            nc.tensor.matmul(
                                ps[:, 0:DL],
                                stg[:, c, sub * 128 : (sub + 1) * 128],
                                wv_sb[:, c, :],
                                start=(c == 0),
                                stop=(c == NC6 - 1),
                            )
                        nc.vector.scalar_tensor_tensor(
                            vsb[:, kt, :, 0:HD],
                            ps[:, 0:DL].rearrange("p (h d) -> p h d", h=HL),
                            0.0,
                            bvb[:].rearrange("p (h d) -> p h d", h=HL),
                            ALU.bypass,
                            ALU.add,
                        )

                # k-group-major within each jq: the 4 head streams advance one
                # score-group at a time; jq0's whole sweep precedes jq1's so
                # jq0's output projection overlaps jq1's attention.
                units = []  # (jq, h, g0, n)
                for jq in range(NJQ):
                    for g0 in range(0, NKT, GRP):
                        n = min(GRP, NKT - g0)
                        for h in range(HL):
                            units.append((jq, h, g0, n))

                pt_t = {}   # unit idx -> pt tile
                sched = {}  # iteration -> list of callables

                def norm_fn(jq, h):
                    def run():
                        qs = slice(jq * 512, (jq + 1) * 512)
                        ss = nz_p.tile([1, 512], F32, tag="nz", name="ss")
                        nc.vector.tensor_copy(ss[:], posb[HD : HD + 1, jq, h, :])
                        rs = nz_p.tile([1, 512], F32, tag="nz", name="rs")
                        nc.vector.reciprocal_approx_fast(rs[:], ss[:])
                        rb = nz_p.tile([HD, 512], F32, tag="nz", name="rb")
                        nc.gpsimd.partition_broadcast(rb[:], rs[:])
                        nc.vector.tensor_mul(
                            o2n[:, h, qs], posb[0:HD, jq, h, :], rb[:]
                        )
                    return run

                def outproj_fn(jq, e):
                    def run():
                        qs = slice(jq * 512, (jq + 1) * 512)
                        pf = pr_ps.tile([128, 512], F32, tag="pr", name="pf")
                        for ht in range(4):
                            nc.tensor.matmul(
                                pf[:],
                                wo_sb[:, ht, e * 128 : (e + 1) * 128],
                                o2n[:, ht, qs],
                                start=(ht == 0),
                                stop=(ht == 3),
                            )
                        osb = nz_p.tile([128, 512], F32, tag="nz", name="osb")
                        nc.vector.tensor_copy(osb[:], pf[:])
                        nc.sync.dma_start(dout[e * 128 : (e + 1) * 128, qs], osb[:])
                    return run

                UN = len(units)
                v_emitted = 0
                for i in range(UN + 12):
                    # 0) v-projection tiles, just ahead of the attn@v that
                    #    needs them (PE fills ACT-bound exp windows)
                    if i < UN:
                        jq, h, g0, n = units[i]
                        jt_need = (g0 + n - 1) // 4
                        while v_emitted <= jt_need:
                            vproj_jt(v_emitted)
                            v_emitted += 1
                    # 1) scores + exp for unit i
                    if i < UN:
                        jq, h, g0, n = units[i]
                        qs = slice(jq * 512, (jq + 1) * 512)
                        sg = s_ps.tile([128, GRP * 512], F32, tag="sg")
                        for t in range(n):
                            kt = g0 + t
                            nc.tensor.matmul(
                                sg[:, t * 512 : (t + 1) * 512],
                                kT[:, h, kt * 128 : (kt + 1) * 128],
                                qT[:, h, qs],
                                start=True,
                                stop=True,
                            )
                        pt = pt_p.tile([128, GRP * 512], BF, tag="pt")
                        nc.scalar.activation(
                            pt[:, : n * 512], sg[:, : n * 512], AF.Exp, scale=SCALE
                        )
                        pt_t[i] = pt
                    # 2) attn@v for unit i-1 into a psum partial, then
                    #    accumulate into the SBUF accumulator on DVE
                    if 1 <= i <= UN:
                        jq, h, g0, n = units[i - 1]
                        pt = pt_t.pop(i - 1)
                        pr = pr_ps.tile([HD + 1, 512], F32, tag="pr", name="pr")
                        for t in range(n):
                            kt = g0 + t
                            nc.tensor.matmul(
                                pr[:],
                                vsb[:, kt, h, :],
                                pt[:, t * 512 : (t + 1) * 512],
                                start=(t == 0),
                                stop=(t == n - 1),
                            )
                        acc = posb[:, jq, h, :]
                        if g0 == 0:
                            nc.vector.tensor_copy(acc, pr[:])
                        else:
                            nc.vector.tensor_add(acc, acc, pr[:])
                        if g0 + n == NKT:
                            sched.setdefault(i + 1, []).append(norm_fn(jq, h))
                            if h == HL - 1:
                                stride = 2 if jq == 0 else 1
                                for e in range(NC6):
                                    sched.setdefault(i + 1 + stride * e, []).append(
                                        outproj_fn(jq, e)
                                    )
                    # 3) deferred normalize / out-projection
                    for fn in sched.pop(i, []):
                        fn()

    nc.compile()
    return nc


def _host_prep(inputs):
    """Build per-core input maps (numpy, bf16-packed)."""
    q = np.asarray(inputs["query"], np.float32)
    k = np.asarray(inputs["key"], np.float32)
    v = np.asarray(inputs["value"], np.float32)
    cq = np.asarray(inputs["coords_query"], np.float32)
    ck = np.asarray(inputs["coords_key"], np.float32)
    Wq = np.asarray(inputs["Wq"], np.float32)
    Wk = np.asarray(inputs["Wk"], np.float32)
    Wv = np.asarray(inputs["Wv"], np.float32)
    Wo = np.asarray(inputs["Wo"], np.float32)
    bq = np.asarray(inputs["bq"], np.float32)
    bk = np.asarray(inputs["bk"], np.float32)
    bv = np.asarray(inputs["bv"], np.float32)

    inv_freq = (
        1.0 / (10000.0 ** (np.arange(16, dtype=np.float32) / np.float32(16.0)))
    ).astype(np.float32)
    ivf = np.zeros((3, HD), np.float32)
    for a in range(3):
        for h2 in range(2):
            ivf[a, a * 32 + h2 * 16 : a * 32 + h2 * 16 + 16] = inv_freq
    p96 = np.zeros((HD, HD), np.float32)
    for a in range(3):
        for j in range(16):
            p96[a * 32 + 16 + j, a * 32 + j] = -1.0
            p96[a * 32 + j, a * 32 + 16 + j] = 1.0

    def pack_tokens(xT, nj):  # xT [768, L] f32 -> [nj*128, 6*512] bf16
        return np.ascontiguousarray(
            xT.reshape(NC6, 128, nj, 512)
            .transpose(2, 1, 0, 3)
            .reshape(nj * 128, NC6 * 512)
        ).astype(BF_NP)

    def pack_w(WT):  # WT [768, 384] f32 -> [128, 6*384] bf16
        return np.ascontiguousarray(
            WT.reshape(NC6, 128, DL).transpose(1, 0, 2).reshape(128, NC6 * DL)
        ).astype(BF_NP)

    in_maps = []
    for c in range(N_CORES):
        b, hg = c // 2, c % 2
        dsl = slice(hg * DL, (hg + 1) * DL)
        woT_loc = np.ascontiguousarray(Wo.T[dsl, :])  # [384, 768]
        wo_pack = np.ascontiguousarray(
            woT_loc.reshape(4, HD, D).transpose(1, 0, 2).reshape(HD, 4 * D)
        ).astype(BF_NP)
        bqh = np.zeros((HD, HL), np.float32)
        bkh = np.zeros((HD, HL), np.float32)
        for h in range(HL):
            bqh[:, h] = bq[hg * DL + h * HD : hg * DL + (h + 1) * HD]
            bkh[:, h] = bk[hg * DL + h * HD : hg * DL + (h + 1) * HD]
        bvb = np.tile(bv[dsl][None, :], (128, 1)).astype(np.float32)
        in_maps.append(
            {
                "qstg": pack_tokens(np.ascontiguousarray(q[b].T), NJQ),
                "kstg": pack_tokens(np.ascontiguousarray(k[b].T), NJK),
                "vstg": pack_tokens(np.ascontiguousarray(v[b].T), NJK),
                "cqT": np.ascontiguousarray(cq[b].T),
                "ckT": np.ascontiguousarray(ck[b].T),
                "wq": pack_w(np.ascontiguousarray(Wq[dsl, :].T)),
                "wk": pack_w(np.ascontiguousarray(Wk[dsl, :].T)),
                "wv": pack_w(np.ascontiguousarray(Wv[dsl, :].T)),
                "wo": wo_pack,
                "bqh": bqh,
                "bkh": bkh,
                "bvb": bvb,
                "ivf": ivf,
                "p96": p96.astype(BF_NP),
            }
        )
    return in_maps


def _run(inputs, trace=False):
    nc = build_program()
    in_maps = _host_prep(inputs)
    res = bass_utils.run_bass_kernel_spmd(
        nc, in_maps, core_ids=list(range(N_CORES)), trace=trace
    )
    bo = np.asarray(inputs["bo"], np.float32)
    out = np.empty((B, Lq, D), np.float32)
    for b in range(B):
        acc = res.results[2 * b]["outT"] + res.results[2 * b + 1]["outT"]
        out[b] = acc.T + bo
    return out, res


def kernel(**inputs) -> np.ndarray:
    out, _ = _run(inputs, trace=False)
    return out


# revision 32
# speedup vs baseline: 1.0291x; 1.0291x over previous
"""Cross-attention layer with 3D RoPE on 8 Trainium2 NeuronCores.

Sharding: core c -> (batch b = c//2, head-group hg = c%2 of 4 heads).
Each core computes its batch's partial output projection for its 4 heads;
the host sums the two half-partials per batch and adds the output bias.

Layout: all heavy tensors are bf16; token/weight inputs are host-packed into
per-partition-concatenated blocks so every big DMA is one fully contiguous
>=0.5MB transfer.  No zero padding: matmuls contract over 96 (head dim) or 3
(coords) partitions directly.

Scheduling: q/k projection skews the rope ops one head behind the projection
matmuls so PE never head-of-line blocks on the ACT bias-add.  Attention runs
k-group-major (all 8 (jq,h) streams advance one score-group at a time) with
the softmax accumulator in SBUF, which lets the v-projection be interleaved
into the attention pipeline (PE does v-proj matmuls while ACT runs exp), and
attn@v weight loads are skewed one group behind the score matmuls.
"""

import sys

import numpy as np
import ml_dtypes

try:
    import concourse.bass as bass
except ImportError:  # fresh-dir grading: container repo path
    sys.path.insert(0, "/opt/trn_rl_repo")
    import concourse.bass as bass

import concourse.tile as tile
from concourse import bacc, bass_utils, mybir

F32 = mybir.dt.float32
R32 = mybir.dt.float32r  # full-rate fp32 matmul mode (1 cyc/row at N>=256)
BF = mybir.dt.bfloat16
BF_NP = ml_dtypes.bfloat16

B, Lq, Lk, D, H = 4, 1024, 4096, 768, 8
HD = 96            # head dim
HL = 4             # heads per core
DL = HL * HD       # 384 local d per core
N_CORES = 8
SCALE = 1.0 / float(np.sqrt(np.float32(HD)))
PI = float(np.pi)

NC6 = D // 128     # 6 c-tiles of the contraction dim
NJQ = Lq // 512    # 2 q token tiles
NJK = Lk // 512    # 8 k token tiles
NKT = Lk // 128    # 32 k tiles for attention
GRP = 3            # score k-tiles per exp instruction


def build_program():
    nc = bacc.Bacc("TRN2", target_bir_lowering=False, debug=False)
    AF = mybir.ActivationFunctionType
    ALU = mybir.AluOpType

    # DRAM I/O (per-core shapes); token/weight blocks are host-packed so each
    # row holds the c-tiles concatenated: row (jt*128+p), col (c*512+t).
    dq = nc.dram_tensor("qstg", [NJQ * 128, NC6 * 512], BF, kind="ExternalInput").ap()
    dk = nc.dram_tensor("kstg", [NJK * 128, NC6 * 512], BF, kind="ExternalInput").ap()
    dv = nc.dram_tensor("vstg", [NJK * 128, NC6 * 512], BF, kind="ExternalInput").ap()
    dcq = nc.dram_tensor("cqT", [3, Lq], R32, kind="ExternalInput").ap()
    dck = nc.dram_tensor("ckT", [3, Lk], R32, kind="ExternalInput").ap()
    dwq = nc.dram_tensor("wq", [128, NC6 * DL], BF, kind="ExternalInput").ap()
    dwk = nc.dram_tensor("wk", [128, NC6 * DL], BF, kind="ExternalInput").ap()
    dwv = nc.dram_tensor("wv", [128, NC6 * DL], BF, kind="ExternalInput").ap()
    dwo = nc.dram_tensor("wo", [HD, 4 * D], BF, kind="ExternalInput").ap()
    dbq = nc.dram_tensor("bqh", [HD, HL], F32, kind="ExternalInput").ap()
    dbk = nc.dram_tensor("bkh", [HD, HL], F32, kind="ExternalInput").ap()
    dbv = nc.dram_tensor("bvb", [128, DL], F32, kind="ExternalInput").ap()
    divf = nc.dram_tensor("ivf", [3, HD], R32, kind="ExternalInput").ap()
    dp96 = nc.dram_tensor("p96", [HD, HD], BF, kind="ExternalInput").ap()
    dout = nc.dram_tensor("outT", [D, Lq], F32, kind="ExternalOutput").ap()

    with tile.TileContext(nc) as tc:
        from contextlib import ExitStack

        with ExitStack() as ctx:
            # ---- persistent tensors ----
            big = ctx.enter_context(tc.tile_pool(name="big", bufs=1))
            kT = big.tile([HD, HL, Lk], BF, tag="kT")
            qT = big.tile([HD, HL, Lq], BF, tag="qT")
            vsb = big.tile([128, NKT, HL, HD + 1], BF, tag="vsb")
            o2n = big.tile([HD, HL, Lq], BF, tag="o2n")
            posb = big.tile([HD + 1, NJQ, HL, 512], F32, tag="posb")

            cst = ctx.enter_context(tc.tile_pool(name="cst", bufs=1))
            wq_sb = cst.tile([128, NC6, DL], BF, tag="wq")
            wk_sb = cst.tile([128, NC6, DL], BF, tag="wk")
            wv_sb = cst.tile([128, NC6, DL], BF, tag="wv")
            wo_sb = cst.tile([HD, 4, D], BF, tag="wo")
            ivf = cst.tile([3, HD], R32, tag="ivf")
            p96 = cst.tile([HD, HD], BF, tag="p96")
            bqh = cst.tile([HD, HL], F32, tag="bqh")
            bkh = cst.tile([HD, HL], F32, tag="bkh")
            bvb = cst.tile([128, DL], F32, tag="bvb")

            stage = ctx.enter_context(tc.tile_pool(name="stage", bufs=4))
            cpool = ctx.enter_context(tc.tile_pool(name="cpool", bufs=2))

            def stage_load(src_dram, jt):
                stg = stage.tile([128, NC6, 512], BF, tag="stg")
                rows = slice(jt * 128, (jt + 1) * 128)
                half = (NC6 // 2) * 512
                nc.sync.dma_start(stg[:, 0 : NC6 // 2, :], src_dram[rows, 0:half])
                nc.sync.dma_start(stg[:, NC6 // 2 : NC6, :], src_dram[rows, half:])
                return stg

            # first-needed first: q-path consts and the first q token tile,
            # then the remaining weights
            nc.sync.dma_start(ivf[:], divf[:])
            nc.sync.dma_start(p96[:], dp96[:])
            nc.sync.dma_start(bqh[:], dbq[:])
            nc.sync.dma_start(wq_sb[:], dwq[:])
            stg0 = stage_load(dq, 0)
            nc.sync.dma_start(wk_sb[:], dwk[:])
            nc.sync.dma_start(bkh[:], dbk[:])
            nc.sync.dma_start(wv_sb[:], dwv[:])
            nc.sync.dma_start(bvb[:], dbv[:])
            nc.sync.dma_start(wo_sb[:], dwo[:])

            # ones column of v (row 96 of the attn@v partial = softmax sum)
            nc.vector.memset(vsb[:, :, :, HD : HD + 1], 1.0)

            with ExitStack() as pctx:
                trig = pctx.enter_context(tc.tile_pool(name="trig", bufs=7))
                tmp = pctx.enter_context(tc.tile_pool(name="tmp", bufs=4))
                th_ps = pctx.enter_context(
                    tc.tile_pool(name="th_ps", bufs=2, space="PSUM")
                )
                pj_ps = pctx.enter_context(
                    tc.tile_pool(name="pj_ps", bufs=2, space="PSUM")
                )
                xs_ps = pctx.enter_context(
                    tc.tile_pool(name="xs_ps", bufs=2, space="PSUM")
                )

                # HAM warm-up: dummy matmuls on a zeroed tile keep the PE busy
                # while the first input DMAs land (cold clock is half rate).
                wrm = trig.tile([HD, 512], R32, tag="trig", name="wrm")
                nc.vector.memset(wrm[:].bitcast(F32), 0.0)
                for i in range(12):
                    wps = th_ps.tile([HD, 512], F32, tag="th", name="wps")
                    nc.tensor.matmul(
                        wps[:], wrm[:, 0:HD], wrm[:], start=True, stop=True
                    )

                # ---- q/k projections + rope, rope skewed one head back ----
                pend = []  # (dst, h, ts, sinD, cosD)

                def do_rope():
                    dst, h, ts, sinD, cosD = pend.pop(0)
                    x = dst[:, h, ts]
                    xs = xs_ps.tile([HD, 512], F32, tag="xs")
                    nc.tensor.matmul(xs[:], p96[:], x, start=True, stop=True)
                    m1 = tmp.tile([HD, 512], BF, tag="m1")
                    nc.gpsimd.tensor_mul(m1[:], x, cosD[:])
                    m2 = tmp.tile([HD, 512], BF, tag="m1", name="m2")
                    nc.vector.tensor_mul(m2[:], xs[:], sinD[:])
                    if h % 2 == 0:
                        nc.vector.tensor_add(x, m1[:], m2[:])
                    else:
                        nc.gpsimd.tensor_add(x, m1[:], m2[:])

                def proj_rope(src_dram, coords_dram, w_sb, bias_sb, dst, n_jt):
                    for jt in range(n_jt):
                        ts = slice(jt * 512, (jt + 1) * 512)
                        if src_dram is dq and jt == 0:
                            stg = stg0
                        else:
                            stg = stage_load(src_dram, jt)
                        # theta -> wrapped -> sin/cos (rows: axis*32+half*16+p)
                        cstg = cpool.tile([3, 512], R32, tag="coords")
                        nc.sync.dma_start(cstg[:], coords_dram[:, ts])
                        th = th_ps.tile([HD, 512], F32, tag="th")
                        nc.tensor.matmul(th[:], ivf[:], cstg[:], start=True, stop=True)
                        ws = trig.tile([HD, 512], F32, tag="trig", name="ws")
                        wc = trig.tile([HD, 512], F32, tag="trig", name="wc")
                        nc.vector.add_range_wrap(ws[:], th[:], 0.0, PI, 2 * PI)
                        nc.vector.add_range_wrap(wc[:], th[:], PI / 2, PI, 2 * PI)
                        sinD = trig.tile([HD, 512], BF, tag="trig", name="sin")
                        cosD = trig.tile([HD, 512], BF, tag="trig", name="cos")
                        nc.scalar.activation(sinD[:], ws[:], AF.Sin)
                        nc.scalar.activation(cosD[:], wc[:], AF.Sin)
                        for h in range(HL):
                            ps = pj_ps.tile([HD, 512], F32, tag="pj")
                            for c in range(NC6):
                                nc.tensor.matmul(
                                    ps[:],
                                    w_sb[:, c, h * HD : (h + 1) * HD],
                                    stg[:, c, :],
                                    start=(c == 0),
                                    stop=(c == NC6 - 1),
                                )
                            nc.scalar.activation(
                                dst[:, h, ts], ps[:], AF.Identity,
                                bias=bias_sb[:, h : h + 1],
                            )
                            pend.append((dst, h, ts, sinD, cosD))
                            if len(pend) > 1:
                                do_rope()

                proj_rope(dq, dcq, wq_sb, bqh, qT, NJQ)
                proj_rope(dk, dck, wk_sb, bkh, kT, NJK)
                while pend:
                    do_rope()

            # ---- attention + v-projection + output, one flat pipeline ----
            with ExitStack() as actx:
                s_ps = actx.enter_context(tc.tile_pool(name="s_ps", bufs=2, space="PSUM"))
                pr_ps = actx.enter_context(tc.tile_pool(name="pr_ps", bufs=2, space="PSUM"))
                pt_p = actx.enter_context(tc.tile_pool(name="pt", bufs=2))
                nz_p = actx.enter_context(tc.tile_pool(name="nz", bufs=4))

                def vproj_jt(jt):
                    stg = stage_load(dv, jt)
                    for sub in range(4):
                        kt = jt * 4 + sub
                        ps = pr_ps.tile([128, 512], F32, tag="pr", name="pv")
                        for c in range(NC6):
                            nc.tensor.matmul(
                                ps[:, 0:DL],
                                stg[:, c, sub * 128 : (sub + 1) * 128],
                                wv_sb[:, c, :],
                                start=(c == 0),
                                stop=(c == NC6 - 1),
                            )
                        nc.vector.scalar_tensor_tensor(
                            vsb[:, kt, :, 0:HD],
                            ps[:, 0:DL].rearrange("p (h d) -> p h d", h=HL),
                            0.0,
                            bvb[:].rearrange("p (h d) -> p h d", h=HL),
                            ALU.bypass,
                            ALU.add,
                        )

                # k-group-major within each jq: the 4 head streams advance one
                # score-group at a time; jq0's whole sweep precedes jq1's so
                # jq0's output projection overlaps jq1's attention.
                units = []  # (jq, h, g0, n)
                for jq in range(NJQ):
                    for g0 in range(0, NKT, GRP):
                        n = min(GRP, NKT - g0)
                        for h in range(HL):
                            units.append((jq, h, g0, n))

                pt_t = {}   # unit idx -> pt tile
                sched = {}  # iteration -> list of callables

                def norm_fn(jq, h):
                    def run():
                        qs = slice(jq * 512, (jq + 1) * 512)
                        ss = nz_p.tile([1, 512], F32, tag="nz", name="ss")
                        nc.vector.tensor_copy(ss[:], posb[HD : HD + 1, jq, h, :])
                        rs = nz_p.tile([1, 512], F32, tag="nz", name="rs")
                        nc.vector.reciprocal_approx_fast(rs[:], ss[:])
                        rb = nz_p.tile([HD, 512], F32, tag="nz", name="rb")
                        nc.gpsimd.partition_broadcast(rb[:], rs[:])
                        nc.vector.tensor_mul(
                            o2n[:, h, qs], posb[0:HD, jq, h, :], rb[:]
                        )
                    return run

                def outproj_fn(jq, e):
                    def run():
                        qs = slice(jq * 512, (jq + 1) * 512)
                        pf = pr_ps.tile([128, 512], F32, tag="pr", name="pf")
                        for ht in range(4):
                            nc.tensor.matmul(
                                pf[:],
                                wo_sb[:, ht, e * 128 : (e + 1) * 128],
                                o2n[:, ht, qs],
                                start=(ht == 0),
                                stop=(ht == 3),
                            )
                        osb = nz_p.tile([128, 512], F32, tag="nz", name="osb")
                        nc.vector.tensor_copy(osb[:], pf[:])
                        nc.sync.dma_start(dout[e * 128 : (e + 1) * 128, qs], osb[:])
                    return run

                UN = len(units)
                v_emitted = 0
                for i in range(UN + 12):
                    # 0) v-projection tiles, just ahead of the attn@v that
                    #    needs them (PE fills ACT-bound exp windows)
                    if i < UN:
                        jq, h, g0, n = units[i]
                        jt_need = (g0 + n - 1) // 4
                        while v_emitted <= jt_need:
                            vproj_jt(v_emitted)
                            v_emitted += 1
                    # 1) scores + exp for unit i
                    if i < UN:
                        jq, h, g0, n = units[i]
                        qs = slice(jq * 512, (jq + 1) * 512)
                        sg = s_ps.tile([128, GRP * 512], F32, tag="sg")
                        for t in range(n):
                            kt = g0 + t
                            nc.tensor.matmul(
                                sg[:, t * 512 : (t + 1) * 512],
                                kT[:, h, kt * 128 : (kt + 1) * 128],
                                qT[:, h, qs],
                                start=True,
                                stop=True,
                            )
                        pt = pt_p.tile([128, GRP * 512], BF, tag="pt")
                        nc.scalar.activation(
                            pt[:, : n * 512], sg[:, : n * 512], AF.Exp, scale=SCALE
                        )
                        pt_t[i] = pt
                    # 2) attn@v for unit i-1 into a psum partial, then
                    #    accumulate into the SBUF accumulator on DVE
                    if 1 <= i <= UN:
                        jq, h, g0, n = units[i - 1]
                        pt = pt_t.pop(i - 1)
                        pr = pr_ps.tile([HD + 1, 512], F32, tag="pr", name="pr")
                        for t in range(n):
                            kt = g0 + t
                            nc.tensor.matmul(
                                pr[:],
                                vsb[:, kt, h, :],
                                pt[:, t * 512 : (t + 1) * 512],
                                start=(t == 0),
                                stop=(t == n - 1),
                            )
                        acc = posb[:, jq, h, :]
                        if g0 == 0:
                            nc.vector.tensor_copy(acc, pr[:])
                        else:
                            nc.vector.tensor_add(acc, acc, pr[:])
                        if g0 + n == NKT:
                            sched.setdefault(i + 1, []).append(norm_fn(jq, h))
                            if h == HL - 1:
                                stride = 2 if jq == 0 else 1
                                for e in range(NC6):
                                    sched.setdefault(i + 2 + stride * e, []).append(
                                        outproj_fn(jq, e)
                                    )
                    # 3) deferred normalize / out-projection
                    for fn in sched.pop(i, []):
                        fn()

    nc.compile()
    return nc


def _host_prep(inputs):
    """Build per-core input maps (numpy, bf16-packed)."""
    q = np.asarray(inputs["query"], np.float32)
    k = np.asarray(inputs["key"], np.float32)
    v = np.asarray(inputs["value"], np.float32)
    cq = np.asarray(inputs["coords_query"], np.float32)
    ck = np.asarray(inputs["coords_key"], np.float32)
    Wq = np.asarray(inputs["Wq"], np.float32)
    Wk = np.asarray(inputs["Wk"], np.float32)
    Wv = np.asarray(inputs["Wv"], np.float32)
    Wo = np.asarray(inputs["Wo"], np.float32)
    bq = np.asarray(inputs["bq"], np.float32)
    bk = np.asarray(inputs["bk"], np.float32)
    bv = np.asarray(inputs["bv"], np.float32)

    inv_freq = (
        1.0 / (10000.0 ** (np.arange(16, dtype=np.float32) / np.float32(16.0)))
    ).astype(np.float32)
    ivf = np.zeros((3, HD), np.float32)
    for a in range(3):
        for h2 in range(2):
            ivf[a, a * 32 + h2 * 16 : a * 32 + h2 * 16 + 16] = inv_freq
    p96 = np.zeros((HD, HD), np.float32)
    for a in range(3):
        for j in range(16):
            p96[a * 32 + 16 + j, a * 32 + j] = -1.0
            p96[a * 32 + j, a * 32 + 16 + j] = 1.0

    def pack_tokens(xT, nj):  # xT [768, L] f32 -> [nj*128, 6*512] bf16
        return np.ascontiguousarray(
            xT.reshape(NC6, 128, nj, 512)
            .transpose(2, 1, 0, 3)
            .reshape(nj * 128, NC6 * 512)
        ).astype(BF_NP)

    def pack_w(WT):  # WT [768, 384] f32 -> [128, 6*384] bf16
        return np.ascontiguousarray(
            WT.reshape(NC6, 128, DL).transpose(1, 0, 2).reshape(128, NC6 * DL)
        ).astype(BF_NP)

    in_maps = []
    for c in range(N_CORES):
        b, hg = c // 2, c % 2
        dsl = slice(hg * DL, (hg + 1) * DL)
        woT_loc = np.ascontiguousarray(Wo.T[dsl, :])  # [384, 768]
        wo_pack = np.ascontiguousarray(
            woT_loc.reshape(4, HD, D).transpose(1, 0, 2).reshape(HD, 4 * D)
        ).astype(BF_NP)
        bqh = np.zeros((HD, HL), np.float32)
        bkh = np.zeros((HD, HL), np.float32)
        for h in range(HL):
            bqh[:, h] = bq[hg * DL + h * HD : hg * DL + (h + 1) * HD]
            bkh[:, h] = bk[hg * DL + h * HD : hg * DL + (h + 1) * HD]
        bvb = np.tile(bv[dsl][None, :], (128, 1)).astype(np.float32)
        in_maps.append(
            {
                "qstg": pack_tokens(np.ascontiguousarray(q[b].T), NJQ),
                "kstg": pack_tokens(np.ascontiguousarray(k[b].T), NJK),
                "vstg": pack_tokens(np.ascontiguousarray(v[b].T), NJK),
                "cqT": np.ascontiguousarray(cq[b].T),
                "ckT": np.ascontiguousarray(ck[b].T),
                "wq": pack_w(np.ascontiguousarray(Wq[dsl, :].T)),
                "wk": pack_w(np.ascontiguousarray(Wk[dsl, :].T)),
                "wv": pack_w(np.ascontiguousarray(Wv[dsl, :].T)),
                "wo": wo_pack,
                "bqh": bqh,
                "bkh": bkh,
                "bvb": bvb,
                "ivf": ivf,
                "p96": p96.astype(BF_NP),
            }
        )
    return in_maps


def _run(inputs, trace=False):
    nc = build_program()
    in_maps = _host_prep(inputs)
    res = bass_utils.run_bass_kernel_spmd(
        nc, in_maps, core_ids=list(range(N_CORES)), trace=trace
    )
    bo = np.asarray(inputs["bo"], np.float32)
    out = np.empty((B, Lq, D), np.float32)
    for b in range(B):
        acc = res.results[2 * b]["outT"] + res.results[2 * b + 1]["outT"]
        out[b] = acc.T + bo
    return out, res


def kernel(**inputs) -> np.ndarray:
    out, _ = _run(inputs, trace=False)
    return out
